# revision 10
# baseline (speedup 1.0000x reference)
"""Bayesian linear layer (reparameterized) on 8 Trainium2 NeuronCores.

y = x @ (mu + exp(log_sigma) * eps_w).T + (bias_mu + exp(bias_log_sigma) * eps_b)

Shapes: x [8192, 4096] f32, weights [16384, 4096] f32, y [8192, 16384] f32.

Strategy (column-parallel over out_features, 2048 outs per core), hybrid
bf16 + fp8 matmul precision:

  - The PE's bf16 roofline for this problem is ~1.75 ms/core. To beat it,
    8 of the 32 contraction k-tiles run as fp8e4 (e4m3) DoubleRow matmuls
    (2 k-tiles per MM at ~1.13x the cost of one bf16 MM). e4m3 noise on
    one quarter of the contraction puts the output at rel-err ~1.9e-2
    (verified bit-exactly against the reference inputs on host), inside
    the 2e-2 budget; the other 24 k-tiles stay bf16.
  - fp8 needs scale folding (e4m3 has no subnormal headroom at these
    magnitudes): x_fp8 = e4m3(8*x), W_fp8 = e4m3(256*W). So that all
    matmuls accumulate into one PSUM group, the bf16-range W is
    host-prescaled by 2048 = 8*256; eviction applies psum*2^-11 + bias in
    a single DVE scalar_tensor_tensor op.
  - Startup: the old layout idled the PE ~130us while the first W columns
    streamed in. Now a narrow 512-col strip 0 is built k-ordered and swept
    k-outer/m-inner in 4-token-tile blocks (4 psum banks per block,
    alternating halves), so the PE chases the W builder a chunk at a
    time. The remaining 1536 cols build during strip 0's sweep, dribbled
    between blocks so the DVE queue never head-of-line blocks evictions.
  - DMA queues: W-build inputs on sync, x tiles on the scalar engine's
    queue, y stores + bias inputs on gpsimd (SWDGE), so stores and W
    streams never stall the latency-critical x loads.
"""

import os
import sys

sys.path.insert(0, "/opt/trn_rl_repo")
os.environ.setdefault("MYCRO_LOCAL_CACHE", "1")

import numpy as np
import ml_dtypes

N_TOK, IN_DIM, OUT_DIM = 8192, 4096, 16384
N_CORES = 8
OUT_S = OUT_DIM // N_CORES  # 2048
P = 128
CW = 512                    # psum-chunk / W-tile width
KF8_T = 8                   # fp8 k-tiles (of 32); must be even
SX = 8.0                    # x fp8 scale
SW = 256.0                  # W fp8 scale
SB = SX * SW                # bf16-range W host prescale
DESCALE = 1.0 / SB


def build_program(n_tok=N_TOK, in_dim=IN_DIM, out_s=OUT_S, n_cores=N_CORES,
                  kf8_t=KF8_T, xt_bufs=8, out_bufs=5, psum_bufs=8):
    """Build + compile the single-core Bass program (SPMD across cores)."""
    import concourse.bass as bass
    import concourse.mybir as mybir
    import concourse.tile as tile
    from concourse import bacc
    from contextlib import ExitStack

    fp32 = mybir.dt.float32
    bf16 = mybir.dt.bfloat16
    fp16 = mybir.dt.float16
    fp8 = mybir.dt.float8e4
    Exp = mybir.ActivationFunctionType.Exp
    add = mybir.AluOpType.add
    mult = mybir.AluOpType.mult
    DR = mybir.MatmulPerfMode.DoubleRow

    KT = in_dim // P
    MT = n_tok // P
    NCH = out_s // CW
    assert in_dim % P == 0 and n_tok % P == 0 and out_s % CW == 0
    assert kf8_t % 2 == 0 and 0 <= kf8_t < KT
    KBF_T = KT - kf8_t          # bf16 k-tiles
    NPAIR = kf8_t // 2          # fp8 DoubleRow pairs
    KF8 = kf8_t * P             # fp8 contraction rows

    nc = bacc.Bacc("TRN2", target_bir_lowering=False, debug=False,
                   num_devices=n_cores, enable_asserts=False)

    # x pre-tiled on host: xB[m, ki, kb, t] = x[m*128 + t, KF8 + kb*128 + ki]
    xB = nc.dram_tensor("xB", [MT, P, KBF_T, P], bf16, kind="ExternalInput")
    if kf8_t:
        # xB8[m, ki, kf, t] = e4m3(8 * x[m*128 + t, kf*128 + ki])
        xB8 = nc.dram_tensor("xB8", [MT, P, kf8_t, P], fp8,
                             kind="ExternalInput")
        muT8 = nc.dram_tensor("muT8", [KF8, out_s], fp16, kind="ExternalInput")
        lsT8 = nc.dram_tensor("lsT8", [KF8, out_s], fp16, kind="ExternalInput")
        epsT8 = nc.dram_tensor("epsT8", [KF8, out_s], fp16,
                               kind="ExternalInput")
    # fp16 inputs: ls ~ -5 and bf16's 8-bit mantissa on ls would be a ~1%
    # multiplicative error after exp; fp16 keeps the bf16-range W at
    # f32-input accuracy at half the DMA traffic. mu/eps carry the 2048x
    # (bf16 range) / 256x (fp8 range) scale from the host.
    muT = nc.dram_tensor("muT", [KBF_T * P, out_s], fp16, kind="ExternalInput")
    lsT = nc.dram_tensor("lsT", [KBF_T * P, out_s], fp16, kind="ExternalInput")
    epsT = nc.dram_tensor("epsT", [KBF_T * P, out_s], fp16,
                          kind="ExternalInput")
    bmu = nc.dram_tensor("bmu", [out_s], fp32, kind="ExternalInput")
    bls = nc.dram_tensor("bls", [out_s], fp32, kind="ExternalInput")
    beps = nc.dram_tensor("beps", [out_s], fp32, kind="ExternalInput")
    y = nc.dram_tensor("y", [n_tok, out_s], fp32, kind="ExternalOutput")

    with tile.TileContext(nc) as tc, ExitStack() as ctx:
        wt_pool = ctx.enter_context(tc.tile_pool(name="wt", bufs=1))
        const_pool = ctx.enter_context(tc.tile_pool(name="const", bufs=1))
        scratch = ctx.enter_context(tc.tile_pool(name="scratch", bufs=2))
        xt_pool = ctx.enter_context(tc.tile_pool(name="xt", bufs=xt_bufs))
        out_pool = ctx.enter_context(tc.tile_pool(name="out", bufs=out_bufs))
        psum_pool = ctx.enter_context(
            tc.tile_pool(name="psum", bufs=psum_bufs, space="PSUM"))

        def fused_w(dst_ap, ls_src, eps_src, mu_src, sync_engine,
                    dt_in, pre="", bufs=None, width=CW):
            # dst = mu + exp(ls) * eps, elementwise over a [P, width] block
            kw = {} if bufs is None else {"bufs": bufs}
            l = scratch.tile([P, width], dt_in, tag=pre + "ls", name="ls_t",
                             **kw)
            e = scratch.tile([P, width], dt_in, tag=pre + "eps", name="eps_t",
                             **kw)
            m_ = scratch.tile([P, width], dt_in, tag=pre + "mu", name="mu_t",
                              **kw)
            x_ = scratch.tile([P, width], fp32, tag=pre + "exp", name="exp_t",
                              **kw)
            sync_engine.dma_start(out=l[:], in_=ls_src)
            sync_engine.dma_start(out=e[:], in_=eps_src)
            sync_engine.dma_start(out=m_[:], in_=mu_src)
            nc.scalar.activation(x_[:], l[:], Exp)
            nc.vector.tensor_mul(x_[:], x_[:], e[:])
            nc.vector.tensor_tensor(dst_ap, x_[:], m_[:], add)

        # bias_rep[p, o] = bmu[o] + exp(bls[o]) * beps[o], natural scale
        bias_rep = const_pool.tile([P, out_s], bf16, tag="bias_rep",
                                   name="bias_rep")

        def bias_chunk(c):
            sl = slice(c * CW, (c + 1) * CW)
            fused_w(bias_rep[:, sl],
                    bls.ap()[sl].partition_broadcast(P),
                    beps.ap()[sl].partition_broadcast(P),
                    bmu.ap()[sl].partition_broadcast(P),
                    nc.gpsimd, fp32, pre="b", bufs=1)

        # ---- W tiles (SBUF-resident for the whole kernel) ----
        wt = {}    # (kb, c) -> [P, CW] bf16
        w8 = {}    # (pj, c) -> [P, 2, CW] fp8

        def build_wt(kb, c):
            t = wt_pool.tile([P, CW], bf16, tag=f"wt{kb}_{c}",
                             name=f"wt{kb}_{c}")
            wt[(kb, c)] = t
            rows = slice(kb * P, (kb + 1) * P)
            sl = slice(c * CW, (c + 1) * CW)
            fused_w(t[:], lsT.ap()[rows, sl], epsT.ap()[rows, sl],
                    muT.ap()[rows, sl], nc.sync, fp16)

        def build_w8(pj, c):
            t = wt_pool.tile([P, 2, CW], fp8, tag=f"w8{pj}_{c}",
                             name=f"w8{pj}_{c}")
            w8[(pj, c)] = t
            sl = slice(c * CW, (c + 1) * CW)
            for i in range(2):
                rows = slice((2 * pj + i) * P, (2 * pj + i + 1) * P)
                fused_w(t[:, i, :], lsT8.ap()[rows, sl],
                        epsT8.ap()[rows, sl], muT8.ap()[rows, sl],
                        nc.sync, fp16)

        def load_x(m):
            # Alternate the bulk x stream between the scalar HWDGE queue
            # and the gpsimd SWDGE queue: one queue tops out ~125 GB/s and
            # strip-0's sweep alone needs ~150 GB/s of x (sync is already
            # carrying the ~100 GB/s W-build stream).
            qa, qb = (nc.scalar, nc.gpsimd) if m % 2 == 0 else \
                     (nc.gpsimd, nc.scalar)
            xt = xt_pool.tile([P, KBF_T, P], bf16, tag="xt", name="xt")
            qa.dma_start(out=xt[:], in_=xB.ap()[m])
            xt8 = None
            if kf8_t:
                xt8 = xt_pool.tile([P, kf8_t, P], fp8, tag="xt8", name="xt8")
                qb.dma_start(out=xt8[:], in_=xB8.ap()[m])
            return xt, xt8

        def k_sweep(psum_of_m, xts, c):
            """Emit the full contraction for psum chunk c over the given
            token tiles. xts: list of (m, xt, xt8). For each k-entity the
            inner loop runs over token tiles (so strip-0 blocks chase the
            W builder chunk by chunk). bf16 first: its first chunk is the
            cheapest build, so the PE starts earliest."""
            for kb in range(KBF_T):
                for m, xt, xt8 in xts:
                    nc.tensor.matmul(
                        psum_of_m[m][:], xt[:, kb, :], wt[(kb, c)][:],
                        start=(kb == 0), stop=(kb == KBF_T - 1 and
                                               NPAIR == 0))
            for pj in range(NPAIR):
                for m, xt, xt8 in xts:
                    nc.tensor.matmul(
                        psum_of_m[m][:], xt8[:, 2 * pj:2 * pj + 2, :],
                        w8[(pj, c)][:], start=(KBF_T == 0 and pj == 0),
                        stop=(pj == NPAIR - 1),
                        perf_mode=DR)

        def evict(ps, m, c):
            ot = out_pool.tile([P, CW], fp32, tag="ot", name="ot")
            # y = psum * 2^-11 + bias in one DVE op; store via SWDGE so the
            # write never head-of-line-blocks a load queue.
            nc.vector.scalar_tensor_tensor(
                ot[:], ps[:], DESCALE, bias_rep[:, c * CW:(c + 1) * CW],
                mult, add)
            nc.gpsimd.dma_start(
                out=y.ap()[m * P:(m + 1) * P, c * CW:(c + 1) * CW],
                in_=ot[:])

        # ---- emission ----
        # Strip-0 builds first, k-ordered: fp8 pairs (cheap, first in the
        # sweep), then the bf16 chunks the blocks will chase.
        x_tiles = {}
        BLK = 4
        blocks = [list(range(m0, min(m0 + BLK, MT)))
                  for m0 in range(0, MT, BLK)]

        for m in blocks[0]:
            x_tiles[m] = load_x(m)

        # Warm-up: throwaway matmuls keep the PE dense through the first
        # W-build window so the HAM clock gate opens to 8/8 and stays.
        if KBF_T >= 2:
            xt0 = x_tiles[blocks[0][0]][0]
            warm_ps = psum_pool.tile([P, CW], fp32, tag="ps", name="warm_ps")
            for _ in range(110):
                nc.tensor.matmul(warm_ps[:, :P], xt0[:, 0, :], xt0[:, 1, :],
                                 start=True, stop=True)

        for kb in range(KBF_T):
            build_wt(kb, 0)
        for pj in range(NPAIR):
            build_w8(pj, 0)
        bias_chunk(0)

        # Remaining strips' builds, dribbled between strip-0 blocks.
        pending = [(lambda pj=pj, c=c: build_w8(pj, c))
                   for c in range(1, NCH) for pj in range(NPAIR)]
        pending += [(lambda kb=kb, c=c: build_wt(kb, c))
                    for c in range(1, NCH) for kb in range(KBF_T)]
        pending += [(lambda c=c: bias_chunk(c)) for c in range(1, NCH)]
        n_pending = len(pending)
        pending = iter(pending)
        per_blk = -(-n_pending // max(len(blocks) - 1, 1))

        # Phase B: strip 0 (chunk 0), k-outer / m-inner per block.
        for bi, blk in enumerate(blocks):
            # prefetch next block's x tiles
            if bi + 1 < len(blocks):
                for m in blocks[bi + 1]:
                    x_tiles[m] = load_x(m)
            psums = {m: psum_pool.tile([P, CW], fp32, tag="ps",
                                       name=f"ps{m}") for m in blk}
            xts = [(m,) + x_tiles[m] for m in blk]
            k_sweep(psums, xts, 0)
            for m in blk:
                evict(psums[m], m, 0)
                del x_tiles[m]
            if bi >= 1:
                for _ in range(per_blk):
                    job = next(pending, None)
                    if job is not None:
                        job()
        for job in pending:
            job()

        # Phase C: strips 1..NCH-1, m-outer with per-m chunk fan-out.
        if NCH > 1:
            for m in range(MT):
                xt, xt8 = load_x(m)
                psc = {c: psum_pool.tile([P, CW], fp32, tag="ps",
                                         name=f"pc{m}_{c}")
                       for c in range(1, NCH)}
                for kb in range(KBF_T):
                    lhsT = xt[:, kb, :]
                    for c in range(1, NCH):
                        nc.tensor.matmul(psc[c][:], lhsT, wt[(kb, c)][:],
                                         start=(kb == 0),
                                         stop=(kb == KBF_T - 1 and
                                               NPAIR == 0))
                for pj in range(NPAIR):
                    lhsT = xt8[:, 2 * pj:2 * pj + 2, :]
                    for c in range(1, NCH):
                        nc.tensor.matmul(psc[c][:], lhsT, w8[(pj, c)][:],
                                         start=(KBF_T == 0 and pj == 0),
                                         stop=(pj == NPAIR - 1),
                                         perf_mode=DR)
                for c in range(1, NCH):
                    evict(psc[c], m, c)

    nc.compile()
    return nc


_PROGRAM_CACHE = {}


def _get_program():
    key = (N_TOK, IN_DIM, OUT_S, KF8_T)
    if key not in _PROGRAM_CACHE:
        _PROGRAM_CACHE[key] = build_program()
    return _PROGRAM_CACHE[key]


def make_in_maps(x, weight_mu, weight_log_sigma, bias_mu, bias_log_sigma,
                 eps_w, eps_b, kf8_t=KF8_T):
    x = np.asarray(x, dtype=np.float32)
    weight_mu = np.asarray(weight_mu, dtype=np.float32)
    weight_log_sigma = np.asarray(weight_log_sigma, dtype=np.float32)
    bias_mu = np.asarray(bias_mu, dtype=np.float32)
    bias_log_sigma = np.asarray(bias_log_sigma, dtype=np.float32)
    eps_w = np.asarray(eps_w, dtype=np.float32)
    eps_b = np.asarray(eps_b, dtype=np.float32)

    MT, KT = N_TOK // P, IN_DIM // P
    KF8 = kf8_t * P
    xr = x.reshape(MT, P, KT, P)  # [m, t, ko, ki]
    xB = np.ascontiguousarray(
        xr[:, :, kf8_t:, :].transpose(0, 3, 2, 1)).astype(ml_dtypes.bfloat16)
    xB8 = np.ascontiguousarray(
        (xr[:, :, :kf8_t, :] * SX).transpose(0, 3, 2, 1)).astype(
            ml_dtypes.float8_e4m3)
    in_maps = []
    for c in range(N_CORES):
        sl = slice(c * OUT_S, (c + 1) * OUT_S)
        im = {
            "xB": xB,
            "muT": (weight_mu[sl, KF8:] * SB).T.astype(np.float16),
            "lsT": weight_log_sigma[sl, KF8:].T.astype(np.float16),
            "epsT": (eps_w[sl, KF8:] * SB).T.astype(np.float16),
            "bmu": np.ascontiguousarray(bias_mu[sl]),
            "bls": np.ascontiguousarray(bias_log_sigma[sl]),
            "beps": np.ascontiguousarray(eps_b[sl]),
        }
        if kf8_t:
            im["xB8"] = xB8
            im["muT8"] = (weight_mu[sl, :KF8] * SW).T.astype(np.float16)
            im["lsT8"] = weight_log_sigma[sl, :KF8].T.astype(np.float16)
            im["epsT8"] = (eps_w[sl, :KF8] * SW).T.astype(np.float16)
        in_maps.append(im)
    return in_maps


def run(in_maps, trace=False, **kwargs):
    import time
    from concourse.bass_utils import run_bass_kernel_spmd
    nc = _get_program()
    for attempt in range(3):
        try:
            res = run_bass_kernel_spmd(nc, in_maps, list(range(N_CORES)),
                                       trace=trace, **kwargs)
            break
        except Exception:  # transient NRT_EXEC_UNIT_UNRECOVERABLE
            if attempt == 2:
                raise
            time.sleep(15)
    out = np.concatenate([res.results[c]["y"] for c in range(N_CORES)], axis=1)
    return out, res


def kernel(x, weight_mu, weight_log_sigma, bias_mu, bias_log_sigma,
           eps_w, eps_b):
    in_maps = make_in_maps(x, weight_mu, weight_log_sigma, bias_mu,
                           bias_log_sigma, eps_w, eps_b)
    out, _ = run(in_maps, trace=False)
    return out


# revision 12
# speedup vs baseline: 1.1850x; 1.1850x over previous
"""Bayesian linear layer (reparameterized) on 8 Trainium2 NeuronCores.

y = x @ (mu + exp(log_sigma) * eps_w).T + (bias_mu + exp(bias_log_sigma) * eps_b)

Shapes: x [8192, 4096] f32, weights [16384, 4096] f32, y [8192, 16384] f32.

Strategy (column-parallel over out_features, 2048 outs per core), hybrid
bf16 + fp8 matmul precision:

  - The PE's bf16 roofline for this problem is ~1.75 ms/core. To beat it,
    8 of the 32 contraction k-tiles run as fp8e4 (e4m3) DoubleRow matmuls
    (2 k-tiles per MM at ~1.13x the cost of one bf16 MM). e4m3 noise on
    one quarter of the contraction puts the output at rel-err ~1.9e-2
    (verified bit-exactly against the reference inputs on host), inside
    the 2e-2 budget; the other 24 k-tiles stay bf16.
  - fp8 needs scale folding (e4m3 has no subnormal headroom at these
    magnitudes): x_fp8 = e4m3(8*x), W_fp8 = e4m3(256*W). So that all
    matmuls accumulate into one PSUM group, the bf16-range W is
    host-prescaled by 2048 = 8*256; eviction applies psum*2^-11 + bias in
    a single DVE scalar_tensor_tensor op.
  - Startup: the old layout idled the PE ~130us while the first W columns
    streamed in. Now a narrow 512-col strip 0 is built k-ordered and swept
    k-outer/m-inner in 4-token-tile blocks (4 psum banks per block,
    alternating halves), so the PE chases the W builder a chunk at a
    time. The remaining 1536 cols build during strip 0's sweep, dribbled
    between blocks so the DVE queue never head-of-line blocks evictions.
  - DMA queues: W-build inputs on sync, x tiles on the scalar engine's
    queue, y stores + bias inputs on gpsimd (SWDGE), so stores and W
    streams never stall the latency-critical x loads.
"""

import os
import sys

sys.path.insert(0, "/opt/trn_rl_repo")
os.environ.setdefault("MYCRO_LOCAL_CACHE", "1")

import numpy as np
import ml_dtypes

N_TOK, IN_DIM, OUT_DIM = 8192, 4096, 16384
N_CORES = 8
OUT_S = OUT_DIM // N_CORES  # 2048
P = 128
CW = 512                    # psum-chunk / W-tile width
KF8_T = 8                   # fp8 k-tiles (of 32); must be even
SX = 8.0                    # x fp8 scale
SW = 256.0                  # W fp8 scale
SB = SX * SW                # bf16-range W host prescale
DESCALE = 1.0 / SB


def build_program(n_tok=N_TOK, in_dim=IN_DIM, out_s=OUT_S, n_cores=N_CORES,
                  kf8_t=KF8_T, xt_bufs=8, out_bufs=5, psum_bufs=8):
    """Build + compile the single-core Bass program (SPMD across cores)."""
    import concourse.bass as bass
    import concourse.mybir as mybir
    import concourse.tile as tile
    from concourse import bacc
    from contextlib import ExitStack

    fp32 = mybir.dt.float32
    bf16 = mybir.dt.bfloat16
    fp16 = mybir.dt.float16
    fp8 = mybir.dt.float8e4
    Exp = mybir.ActivationFunctionType.Exp
    add = mybir.AluOpType.add
    mult = mybir.AluOpType.mult
    DR = mybir.MatmulPerfMode.DoubleRow

    KT = in_dim // P
    MT = n_tok // P
    NCH = out_s // CW
    assert in_dim % P == 0 and n_tok % P == 0 and out_s % CW == 0
    assert kf8_t % 2 == 0 and 0 <= kf8_t < KT
    KBF_T = KT - kf8_t          # bf16 k-tiles
    NPAIR = kf8_t // 2          # fp8 DoubleRow pairs
    KF8 = kf8_t * P             # fp8 contraction rows

    nc = bacc.Bacc("TRN2", target_bir_lowering=False, debug=False,
                   num_devices=n_cores, enable_asserts=False)

    # x pre-tiled on host: xB[m, ki, kb, t] = x[m*128 + t, KF8 + kb*128 + ki]
    xB = nc.dram_tensor("xB", [MT, P, KBF_T, P], bf16, kind="ExternalInput")
    if kf8_t:
        # xB8[m, ki, kf, t] = e4m3(8 * x[m*128 + t, kf*128 + ki])
        xB8 = nc.dram_tensor("xB8", [MT, P, kf8_t, P], fp8,
                             kind="ExternalInput")
        muT8 = nc.dram_tensor("muT8", [KF8, out_s], fp16, kind="ExternalInput")
        lsT8 = nc.dram_tensor("lsT8", [KF8, out_s], fp16, kind="ExternalInput")
        epsT8 = nc.dram_tensor("epsT8", [KF8, out_s], fp16,
                               kind="ExternalInput")
    # fp16 inputs: ls ~ -5 and bf16's 8-bit mantissa on ls would be a ~1%
    # multiplicative error after exp; fp16 keeps the bf16-range W at
    # f32-input accuracy at half the DMA traffic. mu/eps carry the 2048x
    # (bf16 range) / 256x (fp8 range) scale from the host.
    muT = nc.dram_tensor("muT", [KBF_T * P, out_s], fp16, kind="ExternalInput")
    lsT = nc.dram_tensor("lsT", [KBF_T * P, out_s], fp16, kind="ExternalInput")
    epsT = nc.dram_tensor("epsT", [KBF_T * P, out_s], fp16,
                          kind="ExternalInput")
    bmu = nc.dram_tensor("bmu", [out_s], fp32, kind="ExternalInput")
    bls = nc.dram_tensor("bls", [out_s], fp32, kind="ExternalInput")
    beps = nc.dram_tensor("beps", [out_s], fp32, kind="ExternalInput")
    y = nc.dram_tensor("y", [n_tok, out_s], fp32, kind="ExternalOutput")

    with tile.TileContext(nc) as tc, ExitStack() as ctx:
        wt_pool = ctx.enter_context(tc.tile_pool(name="wt", bufs=1))
        const_pool = ctx.enter_context(tc.tile_pool(name="const", bufs=1))
        scratch = ctx.enter_context(tc.tile_pool(name="scratch", bufs=2))
        xt_pool = ctx.enter_context(tc.tile_pool(name="xt", bufs=xt_bufs))
        out_pool = ctx.enter_context(tc.tile_pool(name="out", bufs=out_bufs))
        psum_pool = ctx.enter_context(
            tc.tile_pool(name="psum", bufs=psum_bufs, space="PSUM"))

        def fused_w(dst_ap, ls_src, eps_src, mu_src, sync_engine,
                    dt_in, pre="", bufs=None, width=CW):
            # dst = mu + exp(ls) * eps, elementwise over a [P, width] block
            kw = {} if bufs is None else {"bufs": bufs}
            l = scratch.tile([P, width], dt_in, tag=pre + "ls", name="ls_t",
                             **kw)
            e = scratch.tile([P, width], dt_in, tag=pre + "eps", name="eps_t",
                             **kw)
            m_ = scratch.tile([P, width], dt_in, tag=pre + "mu", name="mu_t",
                              **kw)
            x_ = scratch.tile([P, width], fp32, tag=pre + "exp", name="exp_t",
                              **kw)
            sync_engine.dma_start(out=l[:], in_=ls_src)
            sync_engine.dma_start(out=e[:], in_=eps_src)
            sync_engine.dma_start(out=m_[:], in_=mu_src)
            nc.scalar.activation(x_[:], l[:], Exp)
            nc.vector.tensor_mul(x_[:], x_[:], e[:])
            nc.vector.tensor_tensor(dst_ap, x_[:], m_[:], add)

        # bias_rep[p, o] = bmu[o] + exp(bls[o]) * beps[o], natural scale
        bias_rep = const_pool.tile([P, out_s], bf16, tag="bias_rep",
                                   name="bias_rep")

        def bias_chunk(c):
            sl = slice(c * CW, (c + 1) * CW)
            fused_w(bias_rep[:, sl],
                    bls.ap()[sl].partition_broadcast(P),
                    beps.ap()[sl].partition_broadcast(P),
                    bmu.ap()[sl].partition_broadcast(P),
                    nc.gpsimd, fp32, pre="b", bufs=1)

        # ---- W tiles (SBUF-resident for the whole kernel) ----
        wt = {}    # (kb, c) -> [P, CW] bf16
        w8 = {}    # (pj, c) -> [P, 2, CW] fp8

        def build_wt(kb, c):
            t = wt_pool.tile([P, CW], bf16, tag=f"wt{kb}_{c}",
                             name=f"wt{kb}_{c}")
            wt[(kb, c)] = t
            rows = slice(kb * P, (kb + 1) * P)
            sl = slice(c * CW, (c + 1) * CW)
            fused_w(t[:], lsT.ap()[rows, sl], epsT.ap()[rows, sl],
                    muT.ap()[rows, sl], nc.sync, fp16)

        def build_w8(pj, c):
            t = wt_pool.tile([P, 2, CW], fp8, tag=f"w8{pj}_{c}",
                             name=f"w8{pj}_{c}")
            w8[(pj, c)] = t
            sl = slice(c * CW, (c + 1) * CW)
            for i in range(2):
                rows = slice((2 * pj + i) * P, (2 * pj + i + 1) * P)
                fused_w(t[:, i, :], lsT8.ap()[rows, sl],
                        epsT8.ap()[rows, sl], muT8.ap()[rows, sl],
                        nc.sync, fp16)

        def load_x(m):
            # Bulk bf16 x on the scalar HWDGE queue (sync carries the W
            # stream; both cap ~125 GB/s). The small fp8 x tiles ride the
            # gpsimd SWDGE queue: they are consumed at the END of each
            # k-sweep, so queueing behind a couple of y stores is harmless.
            xt = xt_pool.tile([P, KBF_T, P], bf16, tag="xt", name="xt")
            nc.scalar.dma_start(out=xt[:], in_=xB.ap()[m])
            xt8 = None
            if kf8_t:
                xt8 = xt_pool.tile([P, kf8_t, P], fp8, tag="xt8", name="xt8")
                nc.gpsimd.dma_start(out=xt8[:], in_=xB8.ap()[m])
            return xt, xt8

        def k_sweep(psum_of_m, xts, c):
            """Emit the full contraction for psum chunk c over the given
            token tiles. xts: list of (m, xt, xt8). For each k-entity the
            inner loop runs over token tiles (so strip-0 blocks chase the
            W builder chunk by chunk). bf16 first: its first chunk is the
            cheapest build, so the PE starts earliest."""
            for kb in range(KBF_T):
                for m, xt, xt8 in xts:
                    nc.tensor.matmul(
                        psum_of_m[m][:], xt[:, kb, :], wt[(kb, c)][:],
                        start=(kb == 0), stop=(kb == KBF_T - 1 and
                                               NPAIR == 0))
            for pj in range(NPAIR):
                for m, xt, xt8 in xts:
                    nc.tensor.matmul(
                        psum_of_m[m][:], xt8[:, 2 * pj:2 * pj + 2, :],
                        w8[(pj, c)][:], start=(KBF_T == 0 and pj == 0),
                        stop=(pj == NPAIR - 1),
                        perf_mode=DR)

        def evict(ps, m, c):
            ot = out_pool.tile([P, CW], fp32, tag="ot", name="ot")
            # y = psum * 2^-11 + bias in one DVE op; store via SWDGE so the
            # write never head-of-line-blocks a load queue.
            nc.vector.scalar_tensor_tensor(
                ot[:], ps[:], DESCALE, bias_rep[:, c * CW:(c + 1) * CW],
                mult, add)
            nc.gpsimd.dma_start(
                out=y.ap()[m * P:(m + 1) * P, c * CW:(c + 1) * CW],
                in_=ot[:])

        # ---- emission ----
        # Width cascade, sized to the ~125 GB/s per-DGE-queue cap:
        #   Stage A: m 0..A_M-1 sweep chunk 0 only (narrow: the PE chases
        #     the k-ordered W builder from the first chunk, and A_M is
        #     sized so stage A's PE time covers the strip-0 + strip-1
        #     input stream on the sync queue).
        #   Stage B: remaining m sweep chunks 0+1 (x demand per PE-second
        #     halves, fitting under the scalar queue's cap).
        #   Stage C: chunks 1..3 for stage-A m's, 2..3 for stage-B m's.
        x_tiles = {}
        A_M = MT if NCH < 2 else min(32, MT)
        a_blocks = [list(range(m0, min(m0 + 4, A_M)))
                    for m0 in range(0, A_M, 4)]
        b_blocks = [list(range(m0, min(m0 + 2, MT)))
                    for m0 in range(A_M, MT, 2)]

        for m in a_blocks[0]:
            x_tiles[m] = load_x(m)

        # Warm-up: throwaway matmuls keep the PE dense through the first
        # W-build window so the HAM clock gate opens to 8/8 and stays.
        if KBF_T >= 2:
            xt0 = x_tiles[a_blocks[0][0]][0]
            warm_ps = psum_pool.tile([P, CW], fp32, tag="ps", name="warm_ps")
            for _ in range(110):
                nc.tensor.matmul(warm_ps[:, :P], xt0[:, 0, :], xt0[:, 1, :],
                                 start=True, stop=True)

        # Strip-0 builds, k-ordered. The w8 pairs go after the first few
        # bf16 chunks: early enough that no block stalls at the DR pairs
        # ending its k-sweep, late enough not to delay the first matmul.
        EARLY = min(6, KBF_T)

        def strip_jobs(c):
            jobs = [(lambda kb=kb, c=c: build_wt(kb, c))
                    for kb in range(EARLY)]
            jobs += [(lambda pj=pj, c=c: build_w8(pj, c))
                     for pj in range(NPAIR)]
            jobs += [(lambda kb=kb, c=c: build_wt(kb, c))
                     for kb in range(EARLY, KBF_T)]
            jobs.append(lambda c=c: bias_chunk(c))
            return jobs

        for job in strip_jobs(0):
            job()

        # Strip 1 dribbles through stage A; strips 2+ through stage B.
        jobs_a = strip_jobs(1) if NCH >= 2 else []
        jobs_b = [j for c in range(2, NCH) for j in strip_jobs(c)]
        if not b_blocks:
            jobs_a += jobs_b
            jobs_b = []

        def dribble(it, n):
            for _ in range(n):
                job = next(it, None)
                if job is not None:
                    job()

        # Stage A: chunk 0, k-outer / m-inner blocks of 4.
        pending_a = iter(jobs_a)
        per_a = -(-len(jobs_a) // max(len(a_blocks) - 1, 1))
        for bi, blk in enumerate(a_blocks):
            nxt = a_blocks[bi + 1] if bi + 1 < len(a_blocks) else \
                (b_blocks[0] if b_blocks else None)
            if nxt:
                for m in nxt:
                    x_tiles[m] = load_x(m)
            psums = {m: psum_pool.tile([P, CW], fp32, tag="ps",
                                       name=f"ps{m}") for m in blk}
            k_sweep(psums, [(m,) + x_tiles[m] for m in blk], 0)
            for m in blk:
                evict(psums[m], m, 0)
                del x_tiles[m]
            if bi >= 1:
                dribble(pending_a, per_a)
        dribble(pending_a, len(jobs_a))

        # Stage B: chunks 0+1, 2-m blocks (4 psum banks, halves alternate).
        pending_b = iter(jobs_b)
        per_b = -(-len(jobs_b) // max(len(b_blocks) - 1, 1)) if b_blocks \
            else 0
        for bi, blk in enumerate(b_blocks):
            if bi + 1 < len(b_blocks):
                for m in b_blocks[bi + 1]:
                    x_tiles[m] = load_x(m)
            ps = {(m, c): psum_pool.tile([P, CW], fp32, tag="ps",
                                         name=f"pb{m}_{c}")
                  for m in blk for c in (0, 1)}
            for kb in range(KBF_T):
                for m in blk:
                    lhsT = x_tiles[m][0][:, kb, :]
                    for c in (0, 1):
                        nc.tensor.matmul(ps[(m, c)][:], lhsT, wt[(kb, c)][:],
                                         start=(kb == 0),
                                         stop=(kb == KBF_T - 1 and
                                               NPAIR == 0))
            for pj in range(NPAIR):
                for m in blk:
                    lhsT = x_tiles[m][1][:, 2 * pj:2 * pj + 2, :]
                    for c in (0, 1):
                        nc.tensor.matmul(ps[(m, c)][:], lhsT, w8[(pj, c)][:],
                                         start=(KBF_T == 0 and pj == 0),
                                         stop=(pj == NPAIR - 1),
                                         perf_mode=DR)
            for m in blk:
                for c in (0, 1):
                    evict(ps[(m, c)], m, c)
                del x_tiles[m]
            if bi >= 1:
                dribble(pending_b, per_b)
        dribble(pending_b, len(jobs_b))

        # Stage C: remaining chunks, m-outer with per-m chunk fan-out.
        for m in range(MT):
            lo = 1 if m < A_M else 2
            if lo >= NCH:
                continue
            xt, xt8 = load_x(m)
            psc = {c: psum_pool.tile([P, CW], fp32, tag="ps",
                                     name=f"pc{m}_{c}")
                   for c in range(lo, NCH)}
            for kb in range(KBF_T):
                lhsT = xt[:, kb, :]
                for c in range(lo, NCH):
                    nc.tensor.matmul(psc[c][:], lhsT, wt[(kb, c)][:],
                                     start=(kb == 0),
                                     stop=(kb == KBF_T - 1 and NPAIR == 0))
            for pj in range(NPAIR):
                lhsT = xt8[:, 2 * pj:2 * pj + 2, :]
                for c in range(lo, NCH):
                    nc.tensor.matmul(psc[c][:], lhsT, w8[(pj, c)][:],
                                     start=(KBF_T == 0 and pj == 0),
                                     stop=(pj == NPAIR - 1),
                                     perf_mode=DR)
            for c in range(lo, NCH):
                evict(psc[c], m, c)

    nc.compile()
    return nc


_PROGRAM_CACHE = {}


def _get_program():
    key = (N_TOK, IN_DIM, OUT_S, KF8_T)
    if key not in _PROGRAM_CACHE:
        _PROGRAM_CACHE[key] = build_program()
    return _PROGRAM_CACHE[key]


def make_in_maps(x, weight_mu, weight_log_sigma, bias_mu, bias_log_sigma,
                 eps_w, eps_b, kf8_t=KF8_T):
    x = np.asarray(x, dtype=np.float32)
    weight_mu = np.asarray(weight_mu, dtype=np.float32)
    weight_log_sigma = np.asarray(weight_log_sigma, dtype=np.float32)
    bias_mu = np.asarray(bias_mu, dtype=np.float32)
    bias_log_sigma = np.asarray(bias_log_sigma, dtype=np.float32)
    eps_w = np.asarray(eps_w, dtype=np.float32)
    eps_b = np.asarray(eps_b, dtype=np.float32)

    MT, KT = N_TOK // P, IN_DIM // P
    KF8 = kf8_t * P
    xr = x.reshape(MT, P, KT, P)  # [m, t, ko, ki]
    xB = np.ascontiguousarray(
        xr[:, :, kf8_t:, :].transpose(0, 3, 2, 1)).astype(ml_dtypes.bfloat16)
    xB8 = np.ascontiguousarray(
        (xr[:, :, :kf8_t, :] * SX).transpose(0, 3, 2, 1)).astype(
            ml_dtypes.float8_e4m3)
    in_maps = []
    for c in range(N_CORES):
        sl = slice(c * OUT_S, (c + 1) * OUT_S)
        im = {
            "xB": xB,
            "muT": (weight_mu[sl, KF8:] * SB).T.astype(np.float16),
            "lsT": weight_log_sigma[sl, KF8:].T.astype(np.float16),
            "epsT": (eps_w[sl, KF8:] * SB).T.astype(np.float16),
            "bmu": np.ascontiguousarray(bias_mu[sl]),
            "bls": np.ascontiguousarray(bias_log_sigma[sl]),
            "beps": np.ascontiguousarray(eps_b[sl]),
        }
        if kf8_t:
            im["xB8"] = xB8
            im["muT8"] = (weight_mu[sl, :KF8] * SW).T.astype(np.float16)
            im["lsT8"] = weight_log_sigma[sl, :KF8].T.astype(np.float16)
            im["epsT8"] = (eps_w[sl, :KF8] * SW).T.astype(np.float16)
        in_maps.append(im)
    return in_maps


def run(in_maps, trace=False, **kwargs):
    import time
    from concourse.bass_utils import run_bass_kernel_spmd
    nc = _get_program()
    for attempt in range(3):
        try:
            res = run_bass_kernel_spmd(nc, in_maps, list(range(N_CORES)),
                                       trace=trace, **kwargs)
            break
        except Exception:  # transient NRT_EXEC_UNIT_UNRECOVERABLE
            if attempt == 2:
                raise
            time.sleep(15)
    out = np.concatenate([res.results[c]["y"] for c in range(N_CORES)], axis=1)
    return out, res


def kernel(x, weight_mu, weight_log_sigma, bias_mu, bias_log_sigma,
           eps_w, eps_b):
    in_maps = make_in_maps(x, weight_mu, weight_log_sigma, bias_mu,
                           bias_log_sigma, eps_w, eps_b)
    out, _ = run(in_maps, trace=False)
    return out
